# revision 8
# baseline (speedup 1.0000x reference)
"""Trainium2 Bass kernel for DigitConvolutionalModel.

Conv folded into FC1 on host (W1e = C @ w1), pure data parallel over 8
NeuronCores, feature-major transposed activations, f16 matmul operands with
fp32 PSUM (~4e-4 rel err vs fp32 reference; gate 2e-2).

PE stream-slot minimization (one slot = one 512-col matmul stream ~173 ns):
  * L1 contraction 6x128 + 16-tail; m-chunks [128,128,44] (128-col
    stationaries enable fast weight load).
  * Both subtiles' j0/j1 16-row tails in ONE 4-tile row-pack slot
    (row bases 0/32/64/96).
  * FC2_PACK: subtile 1's third m-chunk shifted to PSUM/a1 partitions
    64-107 so the two 44-row FC2 tails row-pack (strips 0-1 vs 2-3).
  * TAIL_FUSE: the j2 16-row x-tails ride inside the FC2 tail-pack slot
    (FC2 outputs at PSUM col bases 64/32; four disjoint rectangles).
  * Tails issue first (start=True) so m-chunks close right after their
    full-chunk matmuls and ACT evictions pipeline; previous group's FC2
    is deferred mid-group so the PE never waits on an eviction.
  41 slots per 2-tile group = 20.5/tile vs 24 for the original 7x112 layout.
"""

import os
import sys

sys.path.insert(0, "/opt/trn_rl_repo")

import numpy as np

import concourse.tile as tile
from concourse import bacc, mybir
from concourse.bass_utils import run_bass_kernel_spmd

# ---- problem constants ----
B = 65536
D = 784
H = 300
O = 10
IMG = 28
KH = KW = 3
OUT_HW = IMG - KH + 1  # 26

N_CORES = 8
BS = B // N_CORES  # 8192

KF = 6  # full 128-row contraction chunks
KTAIL = D - KF * 128  # 16
BT = 512
NBT = BS // BT  # 16
MPAD = 128

MM_DT = mybir.dt.float16
MM_NP = np.float16

if os.environ.get("M_UNEVEN", "1") == "1":
    M_CHUNKS = [128, 128, 44]  # 128-col stationaries trigger FWL
else:
    M_CHUNKS = [100, 100, 100]
M_OFFS = [sum(M_CHUNKS[:i]) for i in range(3)]
MCH = 3

SUBT = int(os.environ.get("SUBT", "2"))
# FC2_PACK: place subtile 1's third m-chunk at PSUM/a1 partitions 64-107 so
# the two subtiles' 44-row FC2 tail matmuls row-pack into one concurrent slot
FC2_PACK = (
    os.environ.get("FC2_PACK", "1") == "1" and SUBT == 2 and M_CHUNKS[2] == 44
)
# TAIL_FUSE: merge the two subtiles' j0/j1 16-row tails into ONE 4-tile pack
# slot (row bases 0/32/64/96), and ride the j2 16-row tails inside the FC2
# tail-pack slot (FC2 outputs move to PSUM col bases 64/32 so all four
# rectangles are disjoint). 42 -> 41 slots per 2-tile group.
TAIL_FUSE = os.environ.get("TAIL_FUSE", "1") == "1" and FC2_PACK
XP_BUFS = int(os.environ.get("XP_BUFS", "2"))
AP_BUFS = int(os.environ.get("AP_BUFS", "2"))
PS2_BUFS = int(os.environ.get("PS2_BUFS", "2"))
TAIL_MODE = os.environ.get("TAIL_MODE", "pack")  # pack | seq
REPS = int(os.environ.get("KERNEL_REPS", "1"))

_cache = {}


def _build_nc():
    f32 = mybir.dt.float32
    mdt = MM_DT

    nc = bacc.Bacc("TRN2", target_bir_lowering=False, debug=False, num_devices=N_CORES)
    # x full chunks: [128, NBT, KF, BT]; tail replicated at partition bases
    # 0/32/64 with zero gaps: [80, NBT, BT]
    xt_d = nc.declare_dram_parameter("xt", [128, NBT, KF, BT], mdt, isOutput=False)
    XTLP = 112 if TAIL_FUSE else 80
    WTLC = 256 if TAIL_FUSE else 128
    xtl_d = nc.declare_dram_parameter("xtl", [XTLP, NBT, BT], mdt, isOutput=False)
    w1_d = nc.declare_dram_parameter("w1e", [128, KF * H], mdt, isOutput=False)
    wtl_d = nc.declare_dram_parameter("wtl", [XTLP, WTLC], mdt, isOutput=False)
    b1_d = nc.declare_dram_parameter("b1r", [MPAD, MCH], f32, isOutput=False)
    w2_d = nc.declare_dram_parameter("w2r", [MPAD, MCH * O], mdt, isOutput=False)
    b2_d = nc.declare_dram_parameter("b2r", [128, 1], f32, isOutput=False)
    yt_d = nc.declare_dram_parameter("yt", [O, BS], f32, isOutput=True)

    with tile.TileContext(nc) as tc:
        with (
            tc.tile_pool(name="singles", bufs=1) as singles,
            tc.tile_pool(name="xp", bufs=XP_BUFS) as xp,
            tc.tile_pool(name="xtp", bufs=XP_BUFS) as xtp,
            tc.tile_pool(name="ap", bufs=AP_BUFS) as ap,
            tc.tile_pool(name="yp", bufs=3) as yp,
            tc.tile_pool(name="ps1", bufs=(1 if SUBT == 2 else 2), space="PSUM") as ps1p,
            tc.tile_pool(
                name="ps2", bufs=(1 if FC2_PACK else PS2_BUFS), space="PSUM"
            ) as ps2p,
        ):
            w1sb = singles.tile([128, KF * H], mdt)
            nc.sync.dma_start(w1sb[:], w1_d[:])
            wtlsb = singles.tile([128, WTLC], mdt)
            nc.sync.dma_start(wtlsb[0:XTLP, :], wtl_d[:])
            b1sb = singles.tile([MPAD, MCH], f32)
            nc.sync.dma_start(b1sb[:], b1_d[:])
            w2sb = singles.tile([MPAD, MCH * O], mdt)
            nc.sync.dma_start(w2sb[:], w2_d[:])
            b2sb = singles.tile([128, 1], f32)
            nc.sync.dma_start(b2sb[:], b2_d[:])

            def load_x(bt, s):
                xt = xp.tile([128, KF, BT], mdt, name=f"xt{s}")
                nc.sync.dma_start(xt[:], xt_d[:, bt, :, :])
                xtl = xtp.tile([128, BT], mdt, name=f"xtl{s}")
                nc.sync.dma_start(xtl[0:XTLP, :], xtl_d[:, bt, :])
                return xt, xtl

            def layer2_store(a1, bt):
                ps2 = ps2p.tile([O, BT], f32)
                for j in range(MCH):
                    mlen = M_CHUNKS[j]
                    nc.tensor.matmul(
                        ps2[:],
                        w2sb[0:mlen, j * O : (j + 1) * O],
                        a1[0:mlen, j, :],
                        start=(j == 0),
                        stop=(j == MCH - 1),
                    )
                yt = yp.tile([O, BT], f32)
                nc.vector.tensor_scalar_add(yt[:], ps2[:], b2sb[0:O, 0:1])
                nc.sync.dma_start(yt_d[:, bt * BT : (bt + 1) * BT], yt[:])

            def layer2_store_pair(pend, j2tails=None):
                # both subtiles' full chunks interleaved, then the two 44-row
                # tails adjacent: s0 reads a1 rows 0:44 (row strips 0-1), s1
                # reads its shifted copy at rows 64:108 (strips 2-3) ->
                # disjoint row groups, concurrent. With TAIL_FUSE the ps2
                # outputs sit at PSUM col bases 64/32 so the current group's
                # two j2 16-row x-tails ride the same slot at (r0,c0-1) and
                # (r2,c2-3).
                pb = (64, 32) if TAIL_FUSE else (0, 0)
                ps2s = [
                    ps2p.tile([128, BT] if TAIL_FUSE else [O, BT], f32,
                              name=f"ps2_{s}")
                    for s in range(2)
                ]
                outs = [ps2s[s][pb[s] : pb[s] + O, :] for s in range(2)]
                for j in range(2):
                    for s, (a1, bt) in enumerate(pend):
                        nc.tensor.matmul(
                            outs[s],
                            w2sb[0:128, j * O : (j + 1) * O],
                            a1[0:128, j, :],
                            start=(j == 0),
                            stop=False,
                            tile_position=(0, pb[s]) if TAIL_FUSE else None,
                        )
                for s, (a1, bt) in enumerate(pend):
                    mb = 64 * s
                    nc.tensor.matmul(
                        outs[s],
                        w2sb[mb : mb + 44, 2 * O : 3 * O],
                        a1[mb : mb + 44, 2, :],
                        start=False,
                        stop=True,
                        tile_position=(mb, pb[s]) if TAIL_FUSE else None,
                    )
                if j2tails is not None:
                    j2tails()
                for s, (a1, bt) in enumerate(pend):
                    if TAIL_FUSE:
                        yt = yp.tile([128, BT], f32, name=f"ytf{s}")
                        nc.vector.tensor_scalar_add(
                            yt[pb[s] : pb[s] + O, :], outs[s],
                            b2sb[pb[s] : pb[s] + O, 0:1],
                        )
                        nc.sync.dma_start(
                            yt_d[:, bt * BT : (bt + 1) * BT],
                            yt[pb[s] : pb[s] + O, :],
                        )
                    else:
                        yt = yp.tile([O, BT], f32)
                        nc.vector.tensor_scalar_add(yt[:], outs[s], b2sb[0:O, 0:1])
                        nc.sync.dma_start(
                            yt_d[:, bt * BT : (bt + 1) * BT], yt[:]
                        )

            pending = []
            for g in [i for _ in range(REPS) for i in range(NBT // SUBT)]:
                bts = [g * SUBT + s for s in range(SUBT)]
                xts = [load_x(bt, s) for s, bt in enumerate(bts)]
                a1s = [
                    ap.tile([MPAD, MCH, BT], mdt, name=f"a1{s}") for s in range(SUBT)
                ]
                pss = [
                    [ps1p.tile([MPAD, BT], f32, name=f"ps{j}_{s}") for s in range(SUBT)]
                    for j in range(MCH)
                ]
                def j_off(j, s):
                    # FC2_PACK: subtile 1's third m-chunk lives at PSUM/a1
                    # partitions 64-107 so its FC2 tail can row-pack with
                    # subtile 0's
                    return 64 if (FC2_PACK and j == 2 and s == 1) else 0

                # 16-row tails FIRST (start=True): one concurrent row-tiled
                # group per subtile, so each m-chunk closes right after its
                # full-chunk matmuls and the ACT evictions pipeline.
                j2t = None
                if TAIL_FUSE:
                    # one 4-tile slot: j0/j1 tails of both subtiles at row
                    # bases 0/32 (s0, wtl col block A) and 64/96 (s1, block B)
                    for s in range(SUBT):
                        for j in range(2):
                            base = (2 * s + j) * 32
                            wc = 128 * s
                            nc.tensor.matmul(
                                pss[j][s][0 : M_CHUNKS[j], :],
                                wtlsb[base : base + KTAIL, wc : wc + M_CHUNKS[j]],
                                xts[s][1][base : base + KTAIL, :],
                                start=True,
                                stop=False,
                                tile_position=(base, 0),
                            )

                    def j2t(xts=xts, pss=pss):
                        # j2 x-tails ride the FC2 tail-pack slot: s0 at
                        # (r0, c0-1), s1 at (r2, c2-3)
                        nc.tensor.matmul(
                            pss[2][0][0:44, :],
                            wtlsb[0:KTAIL, 128 : 128 + 44],
                            xts[0][1][0:KTAIL, :],
                            start=True,
                            stop=False,
                            tile_position=(0, 0),
                        )
                        nc.tensor.matmul(
                            pss[2][1][64:108, :],
                            wtlsb[64 : 64 + KTAIL, 0:44],
                            xts[1][1][64 : 64 + KTAIL, :],
                            start=True,
                            stop=False,
                            tile_position=(64, 64),
                        )
                else:
                    for s in range(SUBT):
                        for j in range(MCH):
                            base = j * 32
                            mlen = M_CHUNKS[j]
                            off = j_off(j, s)
                            nc.tensor.matmul(
                                pss[j][s][off : off + mlen, :],
                                wtlsb[base : base + KTAIL, 0:mlen],
                                xts[s][1][base : base + KTAIL, :],
                                start=True,
                                stop=False,
                                tile_position=(base, off)
                                if TAIL_MODE == "pack"
                                else None,
                            )
                # full-chunk matmuls: for each (j, k) the SUBT subtiles share
                # one stationary load; m-chunk j closes at k == KF-1.
                for j in range(MCH):
                    mlen, moff = M_CHUNKS[j], M_OFFS[j]
                    for k in range(KF):
                        for s in range(SUBT):
                            off = j_off(j, s)
                            nc.tensor.matmul(
                                pss[j][s][off : off + mlen, :],
                                w1sb[:, k * H + moff : k * H + moff + mlen],
                                xts[s][0][:, k, :],
                                start=False,
                                stop=(k == KF - 1),
                                tile_position=(0, off) if off else None,
                            )
                    for s in range(SUBT):
                        off = j_off(j, s)
                        nc.scalar.activation(
                            a1s[s][off : off + mlen, j, :],
                            pss[j][s][off : off + mlen, :],
                            mybir.ActivationFunctionType.Relu,
                            bias=b1sb[off : off + mlen, j : j + 1],
                        )
                    if j == 0:
                        if pending:
                            # previous group's L2, emitted mid-group so the PE
                            # never waits on that group's last ACT eviction
                            if FC2_PACK and len(pending) == 2:
                                layer2_store_pair(pending, j2tails=j2t)
                            else:
                                for a1p, btp in pending:
                                    layer2_store(a1p, btp)
                            pending = []
                        elif j2t is not None:
                            # first group: no deferred L2 to ride with
                            j2t()
                pending = [(a1s[s], bts[s]) for s in range(SUBT)]
            if FC2_PACK and len(pending) == 2:
                layer2_store_pair(pending)
            else:
                for a1p, btp in pending:
                    layer2_store(a1p, btp)

    nc.compile()
    return nc


def _host_prep_weights(conv_w, w1, b1, w2, b2):
    w1g = w1.astype(np.float64).reshape(OUT_HW, OUT_HW, H)
    w1e = np.zeros((IMG, IMG, H), dtype=np.float64)
    cw = conv_w.astype(np.float64)
    for di in range(KH):
        for dj in range(KW):
            w1e[di : di + OUT_HW, dj : dj + OUT_HW, :] += cw[di, dj] * w1g
    w1e = w1e.reshape(D, H).astype(np.float32)

    w1e_r = np.ascontiguousarray(
        w1e[: KF * 128].reshape(KF, 128, H).transpose(1, 0, 2).reshape(128, KF * H)
    ).astype(MM_NP)
    XTLP = 112 if TAIL_FUSE else 80
    WTLC = 256 if TAIL_FUSE else 128
    wtl = np.zeros((XTLP, WTLC), np.float32)
    for j in range(MCH):
        wtl[j * 32 : j * 32 + KTAIL, 0 : M_CHUNKS[j]] = w1e[
            KF * 128 :, M_OFFS[j] : M_OFFS[j] + M_CHUNKS[j]
        ]
    if TAIL_FUSE:
        # col block B: j2 copy at rows 0:16, j0/j1 copies at rows 64:80/96:112
        wtl[0:KTAIL, 128 : 128 + M_CHUNKS[2]] = w1e[KF * 128 :, M_OFFS[2] :]
        wtl[64 : 64 + KTAIL, 128 : 128 + M_CHUNKS[0]] = w1e[
            KF * 128 :, 0 : M_CHUNKS[0]
        ]
        wtl[96 : 96 + KTAIL, 128 : 128 + M_CHUNKS[1]] = w1e[
            KF * 128 :, M_OFFS[1] : M_OFFS[1] + M_CHUNKS[1]
        ]
    b1f = b1.reshape(H)
    b1_r = np.zeros((MPAD, MCH), np.float32)
    w2_r = np.zeros((MPAD, MCH * O), MM_NP)
    for j in range(MCH):
        mlen, moff = M_CHUNKS[j], M_OFFS[j]
        b1_r[0:mlen, j] = b1f[moff : moff + mlen]
        w2_r[0:mlen, j * O : (j + 1) * O] = w2[moff : moff + mlen, :]
    if M_CHUNKS[2] == 44:
        # replica of the third m-chunk at rows 64:108 for the FC2_PACK path
        b1_r[64:108, 2] = b1f[256:300]
        w2_r[64:108, 2 * O : 3 * O] = w2[256:300, :]
    b2_r = np.zeros((128, 1), np.float32)
    for base in (0, 32, 64):
        b2_r[base : base + O, 0] = b2.reshape(O)
    return w1e_r, wtl.astype(MM_NP), b1_r, w2_r, b2_r


def _host_prep_x(xc):
    """Per-core shard [BS, 784] -> full-chunk + replicated-tail layouts."""
    xc = xc.astype(MM_NP)
    xt = np.ascontiguousarray(
        xc[:, : KF * 128].reshape(NBT, BT, KF, 128).transpose(3, 0, 2, 1)
    )
    tail = xc[:, KF * 128 :].reshape(NBT, BT, KTAIL).transpose(2, 0, 1)  # [16,NBT,BT]
    nrep = 4 if TAIL_FUSE else 3
    xtl = np.zeros((112 if TAIL_FUSE else 80, NBT, BT), MM_NP)
    for j in range(nrep):
        xtl[j * 32 : j * 32 + KTAIL] = tail
    return xt, xtl


def make_in_maps(x, conv_w, w1, b1, w2, b2):
    x = np.asarray(x, dtype=np.float32)
    conv_w = np.asarray(conv_w, np.float32)
    w1 = np.asarray(w1, np.float32)
    b1 = np.asarray(b1, np.float32)
    w2 = np.asarray(w2, np.float32)
    b2 = np.asarray(b2, np.float32)
    w1e_r, wtl, b1_r, w2_r, b2_r = _host_prep_weights(conv_w, w1, b1, w2, b2)
    in_maps = []
    for c in range(N_CORES):
        xt, xtl = _host_prep_x(x[c * BS : (c + 1) * BS])
        in_maps.append(
            {
                "xt": xt,
                "xtl": xtl,
                "w1e": w1e_r,
                "wtl": wtl,
                "b1r": b1_r,
                "w2r": w2_r,
                "b2r": b2_r,
            }
        )
    return in_maps


def build_nc():
    return _build_nc()


def kernel(x, conv_w, w1, b1, w2, b2):
    if "nc" not in _cache:
        _cache["nc"] = build_nc()
    nc = _cache["nc"]

    in_maps = make_in_maps(x, conv_w, w1, b1, w2, b2)
    res = run_bass_kernel_spmd(nc, in_maps, list(range(N_CORES)))

    y = np.empty((B, O), dtype=np.float32)
    for c in range(N_CORES):
        y[c * BS : (c + 1) * BS] = res.results[c]["yt"].T
    return y
